# revision 20
# baseline (speedup 1.0000x reference)
"""LIF spiking-neuron recurrence on Trainium2 (8 NeuronCores), v5.

Reference semantics (TAU=1, THRESH=1, f32):
    mem = 0
    for t in range(T):
        mem = mem + x[t]
        spike[t] = (mem >= 1.0) ? 1.0 : 0.0
        mem = mem * (1 - spike[t])        # hard reset

Sharding: data-parallel over batch (B=128 -> 16 rows/core); per-core
shard viewed as [T, 128, 2048] and host-pre-transposed to [128, T, 2048].

The whole step is ONE custom fused DVE op (registered at import into
concourse's per-NEFF DVE table -- no firmware change, documented
authoring flow in trainium-docs/custom-instructions/04-custom-dve-api.md):

    cleansed = select(prev < -2^99, 0, prev)   # FLAG from last step -> 0
    u        = cleansed + x_t                  # the add (f32, bit-exact)
    out      = select(u < 1, u, FLAG)          # hard reset; FLAG = -2^100

`out` carries the post-reset membrane AND the spike in one f32 tensor:
FLAG (= -2^100) marks "spiked"; legitimate membrane values are bounded
by ~|sum of 64 normals| < 1e3, ~29 orders of magnitude away, and the
next step's cleanse stage maps FLAG back to exactly 0.0 (the hard
reset). Exact ties pre==1.0 take the FLAG branch (u < 1 strict), so
spike=1 -- matching the reference.

Spike extraction is one ACT op off the critical path:
    s8 = Sign(out + 2^99) -> int8 {-1,+1};  host decodes spike = (s8 < 0)
(out=FLAG -> -2^99 -> -1; any legit membrane -> +2^99-ish -> +1).

Per step per core: DVE 1 op (2291ns measured), ACT 1 op (2001ns, leaf),
DMA 1 MiB load (sync HWDGE ring) + 0.25 MiB int8 store (scalar HWDGE
ring -- separate rings so stores never head-of-line block loads). The
kernel is DMA-bound at the traffic floor: ~3.1-3.4us/step at the
measured ~418 GB/s/core. Loads/stores are batched 2 steps per transfer
(chunk=2) with 10 steps of prefetch elasticity -- measured ~25-50us
faster than per-step transfers in same-window A/B under fluctuating
HBM bandwidth. Measured on 8 axon-tunneled trn2 cores: 214-216us HW
exec best (vs 320us for the v1 two-DVE-op kernel), bit-exact vs the
jax f32 reference (rel err 0.0, incl. exact threshold ties).

Why not less: the 84 MB/core of HBM traffic is irreducible without
bit-packing spikes, and any packing pass costs a full extra DVE stream
pass (~2.1us/step -- scan/TT ops are capped at 1 elem/cycle/lane for
this dtype), which exceeds the ~1us/step DVE slack for at most ~7%
DMA savings. SWDGE accum-DMA adds (~8us completion latency on the
recurrence path) and PE identity-matmul adds (f32 = 2 HW passes at
~2x cycles/col + per-matmul LDWEIGHTS) were built, measured, and
rejected -- see git-less lab notes in the session transcript.
"""

import numpy as np

try:
    import concourse  # noqa: F401
except ImportError:  # pragma: no cover
    import sys

    for _p in ("/opt/trn_rl_repo", "/root/.axon_site/_ro/trn_rl_repo"):
        if _p not in sys.path:
            sys.path.insert(0, _p)

from concourse import bacc, mybir
from concourse import dve_ops as _dvo
from concourse.bass_utils import run_bass_kernel_spmd
from concourse.dve_spec import (
    C0,
    C2,
    One,
    Spec,
    Src0,
    Src1,
    Zero,
    _has_src1,
    lower,
    select,
)
from concourse.dve_uop import DveOpSpec
from concourse.mybir import ActivationFunctionType as AF
from concourse.tile import TileContext

T, B, D = 64, 128, 16384
NCORES = 8
BL = B // NCORES  # 16 batch rows per core
P = 128  # SBUF partitions
F = (BL * D) // P  # 2048 free elements per timestep slab

LIF_FLAG = -(2.0**100)   # spike marker in the membrane stream
LIF_CLEAN = -(2.0**99)   # cleanse threshold (membrane can't reach this)
SIGN_BIAS = 2.0**99      # Sign(out + SIGN_BIAS): FLAG -> -1, else +1


def _lif_ref(in0, in1, s0, s1, imm2):
    c = np.where(in0 < s0, np.float32(0.0), in0).astype(np.float32)
    u = (c + in1).astype(np.float32)
    return np.where(u < np.float32(1.0), u, np.float32(imm2)).astype(np.float32)


def _get_lif_op():
    """Register (idempotently) the fused LIF-step DVE op with concourse."""
    name = "LIF_STEP_ANT"
    for op in _dvo.OPS:
        if op.name == name:
            return op
    cleansed = select(Src0 < C0, Zero, Src0)
    u = cleansed + Src1
    spec = Spec(body=select(u < One, u, C2), reference=_lif_ref)
    row = _dvo._CUSTOM_DVE_ROW_BASE + len(_dvo.OPS)
    assert row < 0x20, "out of custom-DVE opcode rows"
    _dvo._SUB_OPCODE_FOR_NAME[name] = row
    shas = {}
    for ver in ("v3", "v4"):
        s = DveOpSpec(
            name=name, opcode=row, uops=lower(spec, ver=ver),
            rd1_en=_has_src1(spec),
        )
        shas[ver] = s.sha(ver)
    op = _dvo.DveOp(name, spec, subdim=False, uops_sha=shas)
    _dvo.OPS.append(op)
    _dvo.CUSTOM_DVE_SPECS[name] = spec
    return op


def build_nc(
    t_steps=T, chunk=2, x_bufs=5, s_bufs=4, store_chunk=None,
    trim_edges=True, alt_rings=False,
):
    """Build + compile the per-core Bass program (identical on all cores).

    Loads and stores are batched `chunk` steps per DMA transfer (bigger
    transfers -> fewer per-transfer overheads and completion gaps); the
    x prefetch pool gives chunk*x_bufs steps of elastic buffering.
    """
    lif = _get_lif_op()
    f32 = mybir.dt.float32
    i8 = mybir.dt.int8
    if store_chunk is None:
        store_chunk = chunk
    assert t_steps % chunk == 0 and chunk % store_chunk == 0
    nc = bacc.Bacc(
        "TRN2", target_bir_lowering=False, debug=False, num_devices=NCORES
    )
    x_ext = nc.dram_tensor("x", [P, t_steps, F], f32, kind="ExternalInput")
    out_ext = nc.dram_tensor("out", [P, t_steps, F], i8, kind="ExternalOutput")
    with TileContext(nc) as tc:
        with (
            tc.tile_pool(name="mp", bufs=1) as mp,
            tc.tile_pool(name="xp", bufs=x_bufs) as xp,
            tc.tile_pool(name="sp", bufs=s_bufs) as sp,
        ):
            b99 = mp.tile([P, 1], f32, name="b99")
            m = [mp.tile([P, F], f32, name=f"m{i}") for i in range(2)]
            nc.vector.memset(b99[:], SIGN_BIAS)
            nc.vector.memset(m[0][:], 0.0)
            n_groups = t_steps // chunk
            for g in range(n_groups):
                # load ring: sync by default; alternate rings if requested
                ld = nc.scalar if (alt_rings and g % 2) else nc.sync
                st = nc.sync if (alt_rings and g % 2) else nc.scalar
                xt = xp.tile([P, chunk, F], f32, name="xt")
                if trim_edges and g == 0:
                    # split the first load so step 0 starts sooner
                    for j in range(chunk):
                        ld.dma_start(xt[:, j, :], x_ext[:, j, :])
                else:
                    ld.dma_start(xt[:], x_ext[:, g * chunk : (g + 1) * chunk, :])
                sgn = sp.tile([P, chunk, F], i8, name="sgn")
                for j in range(chunk):
                    t = g * chunk + j
                    prev = m[t % 2]
                    newm = m[(t + 1) % 2]
                    # the whole LIF step: newm = reset(cleanse(prev) + x_t)
                    nc.vector._custom_dve(
                        lif, out=newm[:], in0=prev[:], in1=xt[:, j, :],
                        s0=LIF_CLEAN, imm2=LIF_FLAG,
                    )
                    # spike: Sign(newm + 2^99) -> int8; leaf off the chain
                    nc.scalar.activation(
                        sgn[:, j, :], newm[:], AF.Sign, bias=b99[:], scale=1.0
                    )
                # stores ride the other HWDGE ring so they never head-of-line
                # block the x-loads (two physical rings)
                sc = 1 if (trim_edges and g == n_groups - 1) else store_chunk
                for k in range(chunk // sc):
                    t0 = g * chunk + k * sc
                    st.dma_start(
                        out_ext[:, t0 : t0 + sc, :].rearrange(
                            "p t f -> p (t f)"
                        ),
                        sgn[:, k * sc : (k + 1) * sc, :]
                        .rearrange("p t f -> p (t f)"),
                    )
    nc.compile()
    return nc


_cached_nc = None


def _get_nc():
    global _cached_nc
    if _cached_nc is None:
        _cached_nc = build_nc()
    return _cached_nc


def _shard(x):
    """Full [T, B, D] -> list of per-core [P, T, F] contiguous arrays."""
    in_maps = []
    for c in range(NCORES):
        xc = x[:, c * BL : (c + 1) * BL, :].reshape(T, P, F).transpose(1, 0, 2)
        in_maps.append({"x": np.ascontiguousarray(xc)})
    return in_maps


def _gather(results):
    """Per-core [P, T, F] int8 sign outputs -> full [T, B, D] f32 spikes."""
    outs = [
        (np.asarray(results[c]["out"]) < 0)
        .astype(np.float32)
        .transpose(1, 0, 2)
        .reshape(T, BL, D)
        for c in range(NCORES)
    ]
    return np.concatenate(outs, axis=1)


def run(x, trace=False, **kw):
    """Run on the 8 NeuronCores; returns (output, BassKernelResults)."""
    x = np.ascontiguousarray(np.asarray(x, dtype=np.float32))
    assert x.shape == (T, B, D), x.shape
    nc = _get_nc()
    res = run_bass_kernel_spmd(
        nc, _shard(x), core_ids=list(range(NCORES)), trace=trace, **kw
    )
    return _gather(res.results), res


def kernel(x: np.ndarray) -> np.ndarray:
    out, _ = run(x)
    return out
